# revision 1
# baseline (speedup 1.0000x reference)
"""Trainium2 Bass kernel for nn_ReasoningLayer (per-token MLP with passthrough pos 0).

Full computation:
  out[:, 0]  = hidden_states[:, 0]
  out[:, i]  = GELU(hidden_states[:, i] @ W1 + b1) @ W2 + b2   for i >= 1

Sharding: pure data parallel over batch — core b computes batch b (2048 tokens).
Device works entirely in transposed layout (x^T [D, T]) so both matmuls have the
contraction dim on SBUF partitions with zero on-device transposes:
  h^T = W1^T-stationary @ x^T   (GELU+b1 fused on ScalarE, per-partition bias)
  y^T = W2^T-stationary @ h^T   (b2 added on VectorE, per-partition scalar)
Host casts x/W to bf16 (fp32 PSUM accumulation) and transposes in/out.
"""

import numpy as np
import ml_dtypes

B, S, D, H = 8, 2048, 1024, 2048
P = 128
NCORES = 8
TCORE = (B * S) // NCORES  # 2048 tokens per core (== one batch)
TSLAB = 512                # tokens per matmul moving-operand slab
NSLAB = TCORE // TSLAB     # 4
DO = D // P                # 8  k-tiles for matmul 1
JO = H // P                # 16 j-tiles (hidden)
OO = D // P                # 8  output tiles

COMPUTE = "bf16"           # "bf16" or "fp32r"

_nc_cache = {}


def _build(compute, repeat=1, tslab=None):
    import concourse.bass as bass
    import concourse.mybir as mybir
    import concourse.tile as tile
    from concourse import bacc

    f32 = mybir.dt.float32
    cdt = mybir.dt.bfloat16 if compute == "bf16" else mybir.dt.float32r
    if tslab is None:
        tslab = 512 if compute == "bf16" else 256
    TSLAB = tslab
    NSLAB = TCORE // TSLAB
    ts = bass.ts
    Gelu = mybir.ActivationFunctionType.Gelu

    nc = bacc.Bacc("TRN2", target_bir_lowering=False, debug=False,
                   num_devices=NCORES)
    xT = nc.dram_tensor("xT", [D, TCORE], cdt, kind="ExternalInput")
    w1 = nc.dram_tensor("w1", [D, H], cdt, kind="ExternalInput")
    b1 = nc.dram_tensor("b1", [H], f32, kind="ExternalInput")
    w2 = nc.dram_tensor("w2", [H, D], cdt, kind="ExternalInput")
    b2 = nc.dram_tensor("b2", [D], f32, kind="ExternalInput")
    yT = nc.dram_tensor("yT", [D, TCORE], f32, kind="ExternalOutput")

    with tile.TileContext(nc) as tc:
        with (
            tc.tile_pool(name="w", bufs=1) as wpool,
            tc.tile_pool(name="bias", bufs=1) as bpool,
            tc.tile_pool(name="x", bufs=2) as xpool,
            tc.tile_pool(name="h", bufs=2) as hpool,
            tc.tile_pool(name="y", bufs=2) as ypool,
            tc.tile_pool(name="ps1", bufs=4, space=bass.MemorySpace.PSUM) as pp1,
            tc.tile_pool(name="ps2", bufs=4, space=bass.MemorySpace.PSUM) as pp2,
        ):
            # Weights, replicated per core. Chunked DMAs spread across queues;
            # j/o-sliced so early matmul groups wait only on the slices they read.
            w1_sb = wpool.tile([P, DO, H], cdt, name="w1_sb")
            w1r = w1.rearrange("(do di) j -> di do j", di=P)
            for jh in range(4):
                for do in range(DO):
                    nc.sync.dma_start(
                        w1_sb[:, do, ts(jh, H // 4)], w1r[:, do, ts(jh, H // 4)]
                    )
            w2_sb = wpool.tile([P, JO, D], cdt, name="w2_sb")
            w2r = w2.rearrange("(jo ji) o -> ji jo o", ji=P)
            for oh in range(2):
                for jo in range(JO):
                    nc.sync.dma_start(
                        w2_sb[:, jo, ts(oh, D // 2)], w2r[:, jo, ts(oh, D // 2)]
                    )
            b1_sb = bpool.tile([P, JO], f32, name="b1_sb")
            nc.sync.dma_start(b1_sb[:], b1.rearrange("(jo ji) -> ji jo", ji=P))
            b2_sb = bpool.tile([P, OO], f32, name="b2_sb")
            nc.sync.dma_start(b2_sb[:], b2.rearrange("(oo oi) -> oi oo", oi=P))

            xTr = xT.rearrange("(do di) t -> di do t", di=P)
            yTr = yT.rearrange("(oo oi) t -> oi oo t", oi=P)

            for it in [i for _ in range(repeat) for i in range(NSLAB)]:
                x_sb = xpool.tile([P, DO, TSLAB], cdt, tag="x_sb")
                for do in range(DO):
                    nc.sync.dma_start(x_sb[:, do], xTr[:, do, ts(it, TSLAB)])

                h_sb = hpool.tile([P, JO, TSLAB], cdt, tag="h_sb")
                for jt in range(JO):
                    ps = pp1.tile([P, TSLAB], f32, tag="ps1")
                    for kt in range(DO):
                        nc.tensor.matmul(
                            ps[:],
                            w1_sb[:, kt, ts(jt, P)],
                            x_sb[:, kt],
                            start=(kt == 0),
                            stop=(kt == DO - 1),
                        )
                    # h^T[j_tile] = Gelu(psum + b1[j_tile])  (bias per partition)
                    nc.scalar.activation(h_sb[:, jt], ps[:], Gelu,
                                         bias=b1_sb[:, ts(jt, 1)])

                y_sb = ypool.tile([P, OO, TSLAB], f32, tag="y_sb")
                for ot in range(OO):
                    ps2 = pp2.tile([P, TSLAB], f32, tag="ps2")
                    for jt in range(JO):
                        nc.tensor.matmul(
                            ps2[:],
                            w2_sb[:, jt, ts(ot, P)],
                            h_sb[:, jt],
                            start=(jt == 0),
                            stop=(jt == JO - 1),
                        )
                    nc.vector.tensor_scalar_add(y_sb[:, ot], ps2[:],
                                                b2_sb[:, ts(ot, 1)])
                for oo in range(OO):
                    nc.sync.dma_start(yTr[:, oo, ts(it, TSLAB)], y_sb[:, oo])

    nc.compile()
    return nc


def _get_nc(compute=COMPUTE, repeat=1, tslab=None):
    key = (compute, repeat, tslab)
    if key not in _nc_cache:
        _nc_cache[key] = _build(compute, repeat, tslab)
    return _nc_cache[key]


def _run(hidden_states, W1, b1, W2, b2, compute=COMPUTE, trace=False):
    from concourse import bass_utils

    nc = _get_nc(compute)
    hidden_states = np.asarray(hidden_states, np.float32)
    cnp = ml_dtypes.bfloat16 if compute == "bf16" else np.float32
    W1c = np.ascontiguousarray(np.asarray(W1).astype(cnp))
    W2c = np.ascontiguousarray(np.asarray(W2).astype(cnp))
    b1c = np.ascontiguousarray(np.asarray(b1, np.float32))
    b2c = np.ascontiguousarray(np.asarray(b2, np.float32))

    in_maps = []
    for c in range(NCORES):
        # order='C' is load-bearing: .T is a strided view and astype's default
        # order='K' would keep it F-ordered, which binds wrong bytes on the
        # native NRT path.
        xT_c = hidden_states[c].T.astype(cnp, order="C")  # [D, TCORE]
        in_maps.append({"xT": xT_c, "w1": W1c, "b1": b1c, "w2": W2c, "b2": b2c})

    res = bass_utils.run_bass_kernel_spmd(
        nc, in_maps, core_ids=list(range(NCORES)), trace=trace
    )

    out = np.empty((B, S, D), np.float32)
    for c in range(NCORES):
        out[c] = res.results[c]["yT"].T
    out[:, 0, :] = hidden_states[:, 0, :]
    return out, res


def kernel(hidden_states, W1, b1, W2, b2):
    out, _ = _run(hidden_states, W1, b1, W2, b2)
    return out

